# revision 37
# baseline (speedup 1.0000x reference)
"""Trainium2 Bass kernel for nn_Actor (blended-MoE actor network).

Computation per batch row b:
    c     = softmax(gate(x_b))                          # [4] blend coeffs
    h1    = relu(sum_e c_e (x_b @ W1_e + b1_e))         # [256]
    h2    = relu(sum_e c_e (h1  @ W2_e + b2_e))         # [128]
    mu    = sum_e c_e (h2 @ Wmu_e + bmu_e)              # [17]

Strategy (pure data-parallel over 8 NeuronCores, 16384 rows/core):
  * Feature-on-partition layout: activations are [feat, batch] tiles, so
    expert weights load directly as matmul lhsT and layer biases are
    per-partition ACT biases.  x is transposed + cast to bf16 on the host.
  * Simplex trick: sum_e c_e A_e = A_3 + sum_{e<3} c_e' (A_e - A_3) with
    c' = c[:3]; saves 1/4 of the per-expert work.
  * Scale-input blending: sum_e c_e (x @ We) = x @ W3 + sum_e ((c_e*x) @ dWe)
    so the expert blend accumulates for free in PSUM.
  * ELU via exact identity elu(z)+1 = relu(z) + min(exp(z), 1); the +1 is
    folded into the next layer's bias on the host.
  * bf16 matmuls with fp32 PSUM accumulation.
"""

import sys

for _p in ("/opt/trn_rl_repo",):
    if _p not in sys.path:
        sys.path.append(_p)

import ml_dtypes
import numpy as np

import concourse.bass as bass
import concourse.mybir as mybir
import concourse.tile as tile
from concourse import bacc
from concourse.bass_utils import run_bass_kernel_spmd

AF = mybir.ActivationFunctionType
BF16 = mybir.dt.bfloat16
F32 = mybir.dt.float32
BF = ml_dtypes.bfloat16

NCORES = 8
B_FULL = 131072
BS = B_FULL // NCORES  # 16384 rows per core
NB = 512               # batch tile (matmul free dim)
D_IN = 256
L1 = 256
L2 = 128
NA = 17
GH = 32


def build_graph(bs: int = BS, num_devices: int = NCORES):
    """Build + compile the per-core Bass graph (same graph on all cores)."""
    nc = bacc.Bacc(
        "TRN2",
        target_bir_lowering=False,
        debug=False,
        enable_asserts=False,
        num_devices=num_devices,
    )
    d = {}

    def din(name, shape, dt):
        d[name] = nc.dram_tensor(name, shape, dt, kind="ExternalInput").ap()

    din("xt", [D_IN, bs], BF16)            # x shard, transposed
    din("w1b", [2, 128, L1], BF16)         # W1[3] as [k, part, m]
    din("w1d", [3, 2, 128, L1], BF16)      # W1[e]-W1[3]
    din("w2b", [2, 128, L2], BF16)
    din("w2d", [3, 2, 128, L2], BF16)
    din("wmub", [128, NA], BF16)
    din("wmud", [3, 128, NA], BF16)
    din("gw1", [2, 128, GH], BF16)
    din("gw2", [GH, GH], BF16)
    din("gwo", [GH, 4], BF16)
    din("b1f", [2, 128, 128], BF16)        # b1 blend weights, K zero-padded to 128
    din("b2f", [128, L2], BF16)
    din("bmuf", [128, NA], BF16)
    din("gb1", [GH, 1], F32)
    din("gb2p", [GH, 1], F32)              # gb2 - colsum(gW2_bf16)
    din("gbop", [4, 1], F32)               # gbo - colsum(gWo_bf16)
    out = nc.dram_tensor("out", [NA, bs], F32, kind="ExternalOutput").ap()

    with tile.TileContext(nc) as tc:
        _body(tc, out, d, bs)
    nc.compile()
    return nc


def _body(tc, out, d, bs):
    nc = tc.nc
    nt = bs // NB

    with (
        tc.tile_pool(name="consts", bufs=1) as consts,
        tc.tile_pool(name="io", bufs=4) as io,
        tc.tile_pool(name="act", bufs=3) as act,
        tc.tile_pool(name="ps_g", bufs=2, space="PSUM") as ps_g,
        tc.tile_pool(name="ps_h1", bufs=2, space="PSUM") as ps_h1,
        tc.tile_pool(name="ps_h2", bufs=2, space="PSUM") as ps_h2,
        tc.tile_pool(name="dram", bufs=3, space="DRAM") as dram,
    ):
        # ---- load constants/weights (once) ----
        w1b_sb = consts.tile([128, 2, L1], BF16)
        w2b_sb = consts.tile([128, 2, L2], BF16)
        for k in range(2):
            nc.sync.dma_start(out=w1b_sb[:, k, :], in_=d["w1b"][k])
            nc.sync.dma_start(out=w2b_sb[:, k, :], in_=d["w2b"][k])
        w1d_sb = consts.tile([128, 3, 2, L1], BF16)
        w2d_sb = consts.tile([128, 3, 2, L2], BF16)
        for e in range(3):
            for k in range(2):
                nc.sync.dma_start(out=w1d_sb[:, e, k, :], in_=d["w1d"][e, k])
                nc.sync.dma_start(out=w2d_sb[:, e, k, :], in_=d["w2d"][e, k])
        wmub_sb = consts.tile([128, NA], BF16)
        nc.sync.dma_start(out=wmub_sb, in_=d["wmub"])
        wmud_sb = consts.tile([128, 3, NA], BF16)
        for e in range(3):
            nc.sync.dma_start(out=wmud_sb[:, e, :], in_=d["wmud"][e])
        gw1_sb = consts.tile([128, 2, GH], BF16)
        for k in range(2):
            nc.sync.dma_start(out=gw1_sb[:, k, :], in_=d["gw1"][k])
        gw2_sb = consts.tile([GH, GH], BF16)
        nc.sync.dma_start(out=gw2_sb, in_=d["gw2"])
        gwo_sb = consts.tile([GH, 4], BF16)
        nc.sync.dma_start(out=gwo_sb, in_=d["gwo"])

        b1f_sb = consts.tile([128, 2, 128], BF16)
        for m in range(2):
            nc.sync.dma_start(out=b1f_sb[:, m, :], in_=d["b1f"][m])
        b2f_sb = consts.tile([128, L2], BF16)
        nc.sync.dma_start(out=b2f_sb, in_=d["b2f"])
        bmuf_sb = consts.tile([128, NA], BF16)
        nc.sync.dma_start(out=bmuf_sb, in_=d["bmuf"])
        gb1_sb = consts.tile([GH, 1], F32)
        nc.sync.dma_start(out=gb1_sb, in_=d["gb1"])
        gb2p_sb = consts.tile([GH, 1], F32)
        nc.sync.dma_start(out=gb2p_sb, in_=d["gb2p"])
        gbop_sb = consts.tile([4, 1], F32)
        nc.sync.dma_start(out=gbop_sb, in_=d["gbop"])

        ones4 = consts.tile([4, 1], BF16)
        nc.vector.memset(ones4, 1.0)

        state = {}

        def gate_phase(t):
            """Load x tile, run gate+softmax, broadcast coeffs, scale x."""
            n0 = t * NB
            xt = io.tile([128, 2, NB], BF16, tag="xt", name=f"xt_{t}")
            nc.sync.dma_start(out=xt[:, 0, :], in_=d["xt"][0:128, n0 : n0 + NB])
            nc.sync.dma_start(out=xt[:, 1, :], in_=d["xt"][128:256, n0 : n0 + NB])
            xt0 = xt[:, 0, :]
            xt1 = xt[:, 1, :]

            # One PSUM bank hosts the whole gate/softmax chain:
            #   base 0:  pg1 then pden;  base 32: pg2 then pr4;  base 64: plg
            G = ps_g.tile([128, NB], F32, tag="g", name=f"G_{t}")
            pg1 = G[0:GH]
            pg2 = G[GH : 2 * GH]
            plg = G[64:68]
            pden = G[0:1]

            def elu1(pg, gb, nm):
                """sbuf bf16 tile = elu(pg + gb) + 1 = relu(z) + min(exp(z),1)"""
                eg = act.tile([GH, NB], BF16, tag=f"eg_{nm}", name=f"eg_{nm}_{t}")
                nc.scalar.activation(eg, pg, AF.Exp, bias=gb)
                rg = act.tile([GH, NB], BF16, tag=f"rg_{nm}", name=f"rg_{nm}_{t}")
                nc.scalar.activation(rg, pg, AF.Relu, bias=gb)
                g = act.tile([GH, NB], BF16, tag=f"g_{nm}", name=f"g_{nm}_{t}")
                nc.vector.scalar_tensor_tensor(
                    g, eg, 1.0, rg,
                    op0=mybir.AluOpType.min, op1=mybir.AluOpType.add,
                )
                return g

            nc.tensor.matmul(pg1, lhsT=gw1_sb[:, 0, :], rhs=xt0, start=True, stop=False)
            nc.tensor.matmul(pg1, lhsT=gw1_sb[:, 1, :], rhs=xt1, start=False, stop=True)
            g1 = elu1(pg1, gb1_sb, "1")
            nc.tensor.matmul(pg2, lhsT=gw2_sb, rhs=g1, start=True, stop=True)
            g2 = elu1(pg2, gb2p_sb, "2")
            nc.tensor.matmul(plg, lhsT=gwo_sb, rhs=g2, start=True, stop=True)
            expv = act.tile([4, NB], BF16, tag="expv", name=f"expv_{t}")
            nc.scalar.activation(expv, plg, AF.Exp, bias=gbop_sb)
            nc.tensor.matmul(pden, lhsT=ones4, rhs=expv, start=True, stop=True)
            rden = act.tile([1, NB], F32, tag="rden", name=f"rden_{t}")
            nc.vector.reciprocal_approx_fast(out=rden, in_=pden)
            # broadcast 1/den to 4 partitions via DRAM bounce (no PE matmul)
            rden_dram = dram.tile([1, NB], F32, tag="rden_dram", name=f"rden_dram_{t}")
            nc.sync.dma_start(out=rden_dram, in_=rden)
            r4sb = act.tile([4, NB], F32, tag="r4sb", name=f"r4sb_{t}")
            nc.sync.dma_start(out=r4sb, in_=rden_dram.to_broadcast([4, NB]))
            # c: [128, NB] zero-padded so bias matmuls can use K=128 tiles
            c = act.tile([128, NB], BF16, tag="c", name=f"c_{t}")
            nc.vector.memset(c, 0.0)
            nc.vector.tensor_mul(c[0:4, :], expv, r4sb)

            # Broadcast c'[0:3] to one [128, 3, 2, NB] tile via DRAM bounce
            # (each row stored twice so one DVE mul covers both k-chunks,
            # and all three experts live in one tile = one PE wait).
            c_dram = dram.tile([1, 3, 2, NB], BF16, tag="c_dram", name=f"c_dram_{t}")
            nc.sync.dma_start(out=c_dram[0:1, :, 0, :], in_=c[0:3, :])
            nc.sync.dma_start(out=c_dram[0:1, :, 1, :], in_=c[0:3, :])
            cb = act.tile([128, 3, 2, NB], BF16, tag="cb", name=f"cb_{t}")
            nc.sync.dma_start(out=cb, in_=c_dram.to_broadcast([128, 3, 2, NB]))

            # y1[e,k] = c'_e * x_k for all (e,k) in ONE DVE op: x is read
            # through a stride-0 "expert" axis.
            xt3 = bass.AP(
                tensor=xt.tensor, offset=xt.offset,
                ap=[list(xt.ap[0]), [0, 3]] + [list(p) for p in xt.ap[1:]],
            )
            y1 = io.tile([128, 3, 2, NB], BF16, tag="y1", name=f"y1_{t}")
            nc.vector.tensor_mul(y1, xt3, cb)
            state[t] = (xt, c, cb, y1)

        def heavy_phase(t):
            n0 = t * NB
            xt, c, cb, y1 = state.pop(t)

            # ---- layer 1 (one [128, 2, NB] PSUM tile spanning 2 banks) ----
            ph1 = ps_h1.tile([128, 2, NB], F32, tag="h1", name=f"ph1_{t}")
            for m in range(2):
                pm = ph1[:, m, :]
                ms = slice(m * 128, (m + 1) * 128)
                nc.tensor.matmul(pm, lhsT=b1f_sb[:, m, :], rhs=c, start=True, stop=False)
                nc.tensor.matmul(pm, lhsT=w1b_sb[:, 0, ms], rhs=xt[:, 0, :], start=False, stop=False)
                nc.tensor.matmul(pm, lhsT=w1b_sb[:, 1, ms], rhs=xt[:, 1, :], start=False, stop=False)
                for e in range(3):
                    for k in range(2):
                        nc.tensor.matmul(
                            pm, lhsT=w1d_sb[:, e, k, ms], rhs=y1[:, e, k, :],
                            start=False, stop=(e == 2 and k == 1),
                        )
            h1 = act.tile([128, 2, NB], BF16, tag="h1", name=f"h1_{t}")
            nc.scalar.activation(h1, ph1, AF.Relu)

            # ---- layer 2 (y2 in one DVE op via stride-0 expert axis) ----
            h13 = bass.AP(
                tensor=h1.tensor, offset=h1.offset,
                ap=[list(h1.ap[0]), [0, 3]] + [list(p) for p in h1.ap[1:]],
            )
            y2 = act.tile([128, 3, 2, NB], BF16, tag="y2", name=f"y2_{t}")
            nc.vector.tensor_mul(y2, h13, cb)
            ph2 = ps_h2.tile([L2, NB], F32, tag="h2", name=f"ph2_{t}")
            nc.tensor.matmul(ph2, lhsT=b2f_sb, rhs=c, start=True, stop=False)
            nc.tensor.matmul(ph2, lhsT=w2b_sb[:, 0, :], rhs=h1[:, 0, :], start=False, stop=False)
            nc.tensor.matmul(ph2, lhsT=w2b_sb[:, 1, :], rhs=h1[:, 1, :], start=False, stop=False)
            for e in range(3):
                for k in range(2):
                    nc.tensor.matmul(
                        ph2, lhsT=w2d_sb[:, e, k, :], rhs=y2[:, e, k, :],
                        start=False, stop=(e == 2 and k == 1),
                    )
            h2 = act.tile([L2, NB], BF16, tag="h2s", name=f"h2_{t}")
            nc.scalar.activation(h2, ph2, AF.Relu)

            # ---- output head (PSUM: reuse partitions 0:17 of ph1 bank 0) ----
            h23 = bass.AP(
                tensor=h2.tensor, offset=h2.offset,
                ap=[list(h2.ap[0]), [0, 3]] + [list(p) for p in h2.ap[1:]],
            )
            y3 = act.tile([L2, 3, NB], BF16, tag="y3", name=f"y3_{t}")
            nc.vector.tensor_mul(y3, h23, cb[:L2, :, 0, :])
            pmu = ph1[0:NA, 0, :]
            nc.tensor.matmul(pmu, lhsT=bmuf_sb, rhs=c, start=True, stop=False)
            nc.tensor.matmul(pmu, lhsT=wmub_sb, rhs=h2, start=False, stop=False)
            for e in range(3):
                nc.tensor.matmul(pmu, lhsT=wmud_sb[:, e, :], rhs=y3[:, e, :], start=False, stop=(e == 2))
            mu = act.tile([NA, NB], F32, tag="mu", name=f"mu_{t}")
            nc.scalar.copy(mu, pmu)
            nc.sync.dma_start(out=out[:, n0 : n0 + NB], in_=mu)

        LEAD = 2
        for i in range(nt + LEAD):
            if i < nt:
                gate_phase(i)
            if i >= LEAD:
                heavy_phase(i - LEAD)


def _padk(b):
    """Pad the expert axis (second-to-last) from 4 to 128 with zeros, bf16."""
    b = np.asarray(b, np.float32)
    shape = list(b.shape)
    shape[-2] = 128
    out = np.zeros(shape, np.float32)
    out[..., :4, :] = b
    return out.astype(BF)


def host_prep(inputs, bs=BS, ncores=NCORES):
    """Convert full f32 inputs to per-core in_maps (weights replicated)."""
    f32 = np.float32
    x = np.asarray(inputs["x"], f32)
    W1 = np.asarray(inputs["W1"], f32)
    b1 = np.asarray(inputs["b1"], f32)
    W2 = np.asarray(inputs["W2"], f32)
    b2 = np.asarray(inputs["b2"], f32)
    Wmu = np.asarray(inputs["Wmu"], f32)
    bmu = np.asarray(inputs["bmu"], f32)
    gW1 = np.asarray(inputs["gW1"], f32)
    gb1 = np.asarray(inputs["gb1"], f32)
    gW2 = np.asarray(inputs["gW2"], f32)
    gb2 = np.asarray(inputs["gb2"], f32)
    gWo = np.asarray(inputs["gWo"], f32)
    gbo = np.asarray(inputs["gbo"], f32)

    gw2_bf = gW2.astype(BF)
    gwo_bf = gWo.astype(BF)
    common = {
        "w1b": W1[3].reshape(2, 128, L1).astype(BF),
        "w1d": (W1[:3] - W1[3]).reshape(3, 2, 128, L1).astype(BF),
        "w2b": W2[3].reshape(2, 128, L2).astype(BF),
        "w2d": (W2[:3] - W2[3]).reshape(3, 2, 128, L2).astype(BF),
        "wmub": Wmu[3].astype(BF),
        "wmud": (Wmu[:3] - Wmu[3]).astype(BF),
        "gw1": gW1.reshape(2, 128, GH).astype(BF),
        "gw2": gw2_bf,
        "gwo": gwo_bf,
        "b1f": _padk(b1.reshape(4, 2, 128).transpose(1, 0, 2)),  # [2, 128, 128]
        "b2f": _padk(b2),                                        # [128, 128]
        "bmuf": _padk(bmu),                                      # [128, 17]
        "gb1": gb1.reshape(GH, 1).astype(f32),
        "gb2p": (gb2 - gw2_bf.astype(f32).sum(0)).reshape(GH, 1).astype(f32),
        "gbop": (gbo - gwo_bf.astype(f32).sum(0)).reshape(4, 1).astype(f32),
    }
    xs = x.reshape(ncores, bs, D_IN)
    in_maps = []
    for i in range(ncores):
        m = dict(common)
        m["xt"] = xs[i].T.astype(BF)
        in_maps.append(m)
    return in_maps


_NC_CACHE = {}


def _get_nc():
    key = (BS, NCORES)
    if key not in _NC_CACHE:
        _NC_CACHE[key] = build_graph(BS, NCORES)
    return _NC_CACHE[key]


def kernel(**inputs):
    in_maps = host_prep(inputs)
    nc = _get_nc()
    res = run_bass_kernel_spmd(nc, in_maps, core_ids=list(range(NCORES)))
    outs = [m["out"] for m in res.results]  # each [17, BS] f32
    return np.concatenate([np.asarray(o, np.float32).T for o in outs], axis=0)


if __name__ == "__main__":
    # smoke build
    nc = build_graph(1024, 1)
    print("built ok")


# revision 60
# speedup vs baseline: 1.1557x; 1.1557x over previous
"""Trainium2 Bass kernel for nn_Actor (blended-MoE actor network).

Computation per batch row b:
    c     = softmax(gate(x_b))                          # [4] blend coeffs
    h1    = relu(sum_e c_e (x_b @ W1_e + b1_e))         # [256]
    h2    = relu(sum_e c_e (h1  @ W2_e + b2_e))         # [128]
    mu    = sum_e c_e (h2 @ Wmu_e + bmu_e)              # [17]

Strategy (pure data-parallel over 8 NeuronCores, 16384 rows/core):
  * Feature-on-partition layout: activations are [feat, batch] tiles, so
    expert weights load directly as matmul lhsT and layer biases are
    per-partition ACT biases.  x is transposed + cast to bf16 on the host.
  * Simplex trick: sum_e c_e A_e = A_3 + sum_{e<3} c_e' (A_e - A_3) with
    c' = c[:3]; saves 1/4 of the per-expert work.
  * Scale-input blending: sum_e c_e (x @ We) = x @ W3 + sum_e ((c_e*x) @ dWe)
    so the expert blend accumulates for free in PSUM.
  * ELU via exact identity elu(z)+1 = relu(z) + min(exp(z), 1); the +1 is
    folded into the next layer's bias on the host.
  * bf16 matmuls with fp32 PSUM accumulation.
"""

import sys

for _p in ("/opt/trn_rl_repo",):
    if _p not in sys.path:
        sys.path.append(_p)

import ml_dtypes
import numpy as np

import concourse.bass as bass
import concourse.mybir as mybir
import concourse.tile as tile
from concourse import bacc
from concourse.bass_utils import run_bass_kernel_spmd

AF = mybir.ActivationFunctionType
BF16 = mybir.dt.bfloat16
F32 = mybir.dt.float32
BF = ml_dtypes.bfloat16

NCORES = 8
B_FULL = 131072
BS = B_FULL // NCORES  # 16384 rows per core
NB = 512               # batch tile (matmul free dim)
D_IN = 256
L1 = 256
L2 = 128
NA = 17
GH = 32


def build_graph(bs: int = BS, num_devices: int = NCORES):
    """Build + compile the per-core Bass graph (same graph on all cores)."""
    nc = bacc.Bacc(
        "TRN2",
        target_bir_lowering=False,
        debug=False,
        enable_asserts=False,
        num_devices=num_devices,
    )
    d = {}

    def din(name, shape, dt):
        d[name] = nc.dram_tensor(name, shape, dt, kind="ExternalInput").ap()

    din("xt", [D_IN, bs], BF16)            # x shard, transposed
    din("w1b", [2, 128, L1], BF16)         # W1[3] as [k, part, m]
    din("w1d", [3, 2, 128, L1], BF16)      # W1[e]-W1[3]
    din("w2b", [2, 128, L2], BF16)
    din("w2d", [3, 2, 128, L2], BF16)
    din("wmub", [128, NA], BF16)
    din("wmud", [3, 128, NA], BF16)
    din("gw1", [2, 128, GH], BF16)
    din("gw2x3", [3 * GH, GH], BF16)       # gate W2 replicated at bases 0/32/64
    din("gwox3", [3 * GH, 4], BF16)
    # bias blend weights, zero-padded to K=128 with the live rows at
    # partition 32v (three variants, one per position in the gate triple)
    din("b1f", [3, 2, 128, 128], BF16)
    din("b2f", [3, 128, L2], BF16)
    din("bmuf", [3, 128, NA], BF16)
    din("gb1", [3 * GH, 1], F32)           # tiled x3
    din("gb2p", [3 * GH, 1], F32)          # (gb2 - colsum(gW2_bf16)) x3
    din("gbop", [3 * GH, 1], F32)          # gbo' at rows 32v..32v+3
    out = nc.dram_tensor("out", [NA, bs], F32, kind="ExternalOutput").ap()

    with tile.TileContext(nc) as tc:
        _body(tc, out, d, bs)
    nc.compile()
    return nc


def _body(tc, out, d, bs, dbg=None):
    nc = tc.nc
    nt = bs // NB

    with (
        tc.tile_pool(name="consts", bufs=1) as consts,
        tc.tile_pool(name="io", bufs=6) as io,
        tc.tile_pool(name="act", bufs=3) as act,
        tc.tile_pool(name="ps_g", bufs=1, space="PSUM") as ps_g,
        tc.tile_pool(name="ps_h1", bufs=2, space="PSUM") as ps_h1,
        tc.tile_pool(name="ps_h2", bufs=2, space="PSUM") as ps_h2,
        tc.tile_pool(name="dram", bufs=3, space="DRAM") as dram,
    ):
        # ---- load constants/weights (once) ----
        w1b_sb = consts.tile([128, 2, L1], BF16)
        w2b_sb = consts.tile([128, 2, L2], BF16)
        for k in range(2):
            nc.sync.dma_start(out=w1b_sb[:, k, :], in_=d["w1b"][k])
            nc.sync.dma_start(out=w2b_sb[:, k, :], in_=d["w2b"][k])
        w1d_sb = consts.tile([128, 3, 2, L1], BF16)
        w2d_sb = consts.tile([128, 3, 2, L2], BF16)
        for e in range(3):
            for k in range(2):
                nc.sync.dma_start(out=w1d_sb[:, e, k, :], in_=d["w1d"][e, k])
                nc.sync.dma_start(out=w2d_sb[:, e, k, :], in_=d["w2d"][e, k])
        wmub_sb = consts.tile([128, NA], BF16)
        nc.sync.dma_start(out=wmub_sb, in_=d["wmub"])
        wmud_sb = consts.tile([128, 3, NA], BF16)
        for e in range(3):
            nc.sync.dma_start(out=wmud_sb[:, e, :], in_=d["wmud"][e])
        gw1_sb = consts.tile([128, 2, GH], BF16)
        for k in range(2):
            nc.sync.dma_start(out=gw1_sb[:, k, :], in_=d["gw1"][k])
        gw2x3_sb = consts.tile([3 * GH, GH], BF16)
        nc.sync.dma_start(out=gw2x3_sb, in_=d["gw2x3"])
        gwox3_sb = consts.tile([3 * GH, 4], BF16)
        nc.sync.dma_start(out=gwox3_sb, in_=d["gwox3"])

        b1f_sb = consts.tile([128, 3, 2, 128], BF16)
        b2f_sb = consts.tile([128, 3, L2], BF16)
        bmuf_sb = consts.tile([128, 3, NA], BF16)
        for v in range(3):
            for m in range(2):
                nc.sync.dma_start(out=b1f_sb[:, v, m, :], in_=d["b1f"][v, m])
            nc.sync.dma_start(out=b2f_sb[:, v, :], in_=d["b2f"][v])
            nc.sync.dma_start(out=bmuf_sb[:, v, :], in_=d["bmuf"][v])
        gb1_sb = consts.tile([3 * GH, 1], F32)
        nc.sync.dma_start(out=gb1_sb, in_=d["gb1"])
        gb2p_sb = consts.tile([3 * GH, 1], F32)
        nc.sync.dma_start(out=gb2p_sb, in_=d["gb2p"])
        gbop_sb = consts.tile([3 * GH, 1], F32)
        nc.sync.dma_start(out=gbop_sb, in_=d["gbop"])

        ones96 = consts.tile([3 * GH, 1], BF16)
        nc.vector.memset(ones96, 1.0)

        # persistent zero-padded coefficient slots: c(t) occupies rows
        # 32(t%3)..+3 of column t%6; all other rows stay zero.  6 columns =
        # two gate-triples deep, since gate_triple(t0+3) is emitted before
        # heavy(t0+1)/heavy(t0+2) have consumed their coefficients.
        c_slots = consts.tile([128, 6, NB], BF16)
        nc.vector.memset(c_slots, 0.0)

        state = {}

        def gate_triple(t0):
            """Gate+softmax for supertiles t0..t0+2, fused across the triple:
            the three gate chains live at partition bases 0/32/64 of shared
            PSUM banks so each ACT/DVE elu op covers all three at once."""
            ts_ = [t for t in (t0, t0 + 1, t0 + 2) if t < nt]
            nm = len(ts_)
            P = GH * nm
            xts = {}
            for j, t in enumerate(ts_):
                n0 = t * NB
                xt = io.tile([128, 2, NB], BF16, tag="xt", name=f"xt_{t}")
                nc.sync.dma_start(out=xt[:, 0, :], in_=d["xt"][0:128, n0 : n0 + NB])
                nc.sync.dma_start(out=xt[:, 1, :], in_=d["xt"][128:256, n0 : n0 + NB])
                xts[t] = xt

            GA = ps_g.tile([128, NB], F32, tag="ga", name=f"GA_{t0}")
            GB = ps_g.tile([128, NB], F32, tag="gb", name=f"GB_{t0}")

            # layer 1 of the gate: pg1(t_j) at GA[32j:32j+32]
            for j, t in enumerate(ts_):
                pg1 = GA[GH * j : GH * (j + 1)]
                nc.tensor.matmul(pg1, lhsT=gw1_sb[:, 0, :], rhs=xts[t][:, 0, :], start=True, stop=False)
                nc.tensor.matmul(pg1, lhsT=gw1_sb[:, 1, :], rhs=xts[t][:, 1, :], start=False, stop=True)
            eg1 = act.tile([P, NB], BF16, tag="eg1", name=f"eg1_{t0}")
            nc.scalar.activation(eg1, GA[0:P], AF.Exp, bias=gb1_sb[0:P])
            rg1 = act.tile([P, NB], BF16, tag="rg1", name=f"rg1_{t0}")
            nc.scalar.activation(rg1, GA[0:P], AF.Relu, bias=gb1_sb[0:P])
            g1 = act.tile([P, NB], BF16, tag="g1", name=f"g1_{t0}")
            nc.vector.scalar_tensor_tensor(
                g1, eg1, 1.0, rg1, op0=mybir.AluOpType.min, op1=mybir.AluOpType.add
            )
            # layer 2: K=32 row-group matmuls run concurrently
            for j in range(nm):
                s = slice(GH * j, GH * (j + 1))
                nc.tensor.matmul(GB[s], lhsT=gw2x3_sb[s, :], rhs=g1[s, :], start=True, stop=True)
            eg2 = act.tile([P, NB], BF16, tag="eg2", name=f"eg2_{t0}")
            nc.scalar.activation(eg2, GB[0:P], AF.Exp, bias=gb2p_sb[0:P])
            rg2 = act.tile([P, NB], BF16, tag="rg2", name=f"rg2_{t0}")
            nc.scalar.activation(rg2, GB[0:P], AF.Relu, bias=gb2p_sb[0:P])
            g2 = act.tile([P, NB], BF16, tag="g2", name=f"g2_{t0}")
            nc.vector.scalar_tensor_tensor(
                g2, eg2, 1.0, rg2, op0=mybir.AluOpType.min, op1=mybir.AluOpType.add
            )
            # logits at GA[32j:32j+4] (reusing GA after eg1/rg1 read it)
            expv = act.tile([GH * 2 + 4, NB], BF16, tag="expv", name=f"expv_{t0}")
            for j, t in enumerate(ts_):
                s4 = slice(GH * j, GH * j + 4)
                nc.tensor.matmul(
                    GA[s4], lhsT=gwox3_sb[GH * j : GH * (j + 1), :],
                    rhs=g2[GH * j : GH * (j + 1), :], start=True, stop=True,
                )
                nc.scalar.activation(expv[s4], GA[s4], AF.Exp, bias=gbop_sb[s4])
                # denominator at GB[0:1] (base 0: reciprocal_approx_fast
                # gives wrong results at nonzero base partitions on HW)
                nc.tensor.matmul(
                    GB[0:1], lhsT=ones96[s4], rhs=expv[s4], start=True, stop=True
                )
                rden = act.tile([1, NB], F32, tag=f"rden{j}", name=f"rden_{t}")
                nc.vector.reciprocal_approx_fast(out=rden, in_=GB[0:1])
                # broadcast 1/den to the 4 coeff rows via DRAM bounce
                rden_dram = dram.tile([1, NB], F32, tag="rden_dram", name=f"rden_dram_{t}")
                nc.sync.dma_start(out=rden_dram, in_=rden)
                r4sb = act.tile([GH * 2 + 4, NB], F32, tag="r4sb", name=f"r4sb_{t}")
                nc.sync.dma_start(out=r4sb[s4], in_=rden_dram.to_broadcast([4, NB]))
                cs = c_slots[:, t % 6, :]
                nc.vector.tensor_mul(cs[s4], expv[s4], r4sb[s4])

                # Broadcast c rows to one [128, 3, 2, NB] tile via DRAM bounce
                c_dram = dram.tile([1, 3, 2, NB], BF16, tag="c_dram", name=f"c_dram_{t}")
                nc.sync.dma_start(out=c_dram[0:1, :, 0, :], in_=cs[GH * j : GH * j + 3, :])
                nc.sync.dma_start(out=c_dram[0:1, :, 1, :], in_=cs[GH * j : GH * j + 3, :])
                cb = act.tile([128, 3, 2, NB], BF16, tag="cb", name=f"cb_{t}", bufs=6)
                nc.sync.dma_start(out=cb, in_=c_dram.to_broadcast([128, 3, 2, NB]))

                # y1[e,k] = c'_e * x_k for all (e,k) in ONE DVE op
                xt = xts[t]
                xt3 = bass.AP(
                    tensor=xt.tensor, offset=xt.offset,
                    ap=[list(xt.ap[0]), [0, 3]] + [list(p) for p in xt.ap[1:]],
                )
                y1 = io.tile([128, 3, 2, NB], BF16, tag="y1", name=f"y1_{t}")
                nc.vector.tensor_mul(y1, xt3, cb)
                state[t] = (xt, cs, cb, y1)

            if dbg is not None and t0 == 0:
                nc.sync.dma_start(out=dbg["g1"], in_=g1)
                nc.sync.dma_start(out=dbg["g2"], in_=g2)
                nc.sync.dma_start(out=dbg["expv"], in_=expv)
                nc.sync.dma_start(out=dbg["c"], in_=c_slots)

        def heavy_phase(t):
            n0 = t * NB
            xt, c, cb, y1 = state.pop(t)
            v = t % 3

            # ---- layer 1 (one [128, 2, NB] PSUM tile spanning 2 banks) ----
            ph1 = ps_h1.tile([128, 2, NB], F32, tag="h1", name=f"ph1_{t}")
            for m in range(2):
                pm = ph1[:, m, :]
                ms = slice(m * 128, (m + 1) * 128)
                nc.tensor.matmul(pm, lhsT=b1f_sb[:, v, m, :], rhs=c, start=True, stop=False)
                nc.tensor.matmul(pm, lhsT=w1b_sb[:, 0, ms], rhs=xt[:, 0, :], start=False, stop=False)
                nc.tensor.matmul(pm, lhsT=w1b_sb[:, 1, ms], rhs=xt[:, 1, :], start=False, stop=False)
                for e in range(3):
                    for k in range(2):
                        nc.tensor.matmul(
                            pm, lhsT=w1d_sb[:, e, k, ms], rhs=y1[:, e, k, :],
                            start=False, stop=(e == 2 and k == 1),
                        )
            h1 = act.tile([128, 2, NB], BF16, tag="h1", name=f"h1_{t}")
            nc.scalar.activation(h1, ph1, AF.Relu)

            # ---- layer 2 (y2 in one DVE op via stride-0 expert axis) ----
            h13 = bass.AP(
                tensor=h1.tensor, offset=h1.offset,
                ap=[list(h1.ap[0]), [0, 3]] + [list(p) for p in h1.ap[1:]],
            )
            y2 = act.tile([128, 3, 2, NB], BF16, tag="y2", name=f"y2_{t}")
            nc.vector.tensor_mul(y2, h13, cb)
            ph2 = ps_h2.tile([L2, NB], F32, tag="h2", name=f"ph2_{t}")
            nc.tensor.matmul(ph2, lhsT=b2f_sb[:, v, :], rhs=c, start=True, stop=False)
            nc.tensor.matmul(ph2, lhsT=w2b_sb[:, 0, :], rhs=h1[:, 0, :], start=False, stop=False)
            nc.tensor.matmul(ph2, lhsT=w2b_sb[:, 1, :], rhs=h1[:, 1, :], start=False, stop=False)
            for e in range(3):
                for k in range(2):
                    nc.tensor.matmul(
                        ph2, lhsT=w2d_sb[:, e, k, :], rhs=y2[:, e, k, :],
                        start=False, stop=(e == 2 and k == 1),
                    )
            h2 = act.tile([L2, NB], BF16, tag="h2s", name=f"h2_{t}")
            nc.scalar.activation(h2, ph2, AF.Relu)

            # ---- output head (PSUM: reuse partitions 0:17 of ph1 bank 0) ----
            h23 = bass.AP(
                tensor=h2.tensor, offset=h2.offset,
                ap=[list(h2.ap[0]), [0, 3]] + [list(p) for p in h2.ap[1:]],
            )
            y3 = act.tile([L2, 3, NB], BF16, tag="y3", name=f"y3_{t}")
            nc.vector.tensor_mul(y3, h23, cb[:L2, :, 0, :])
            pmu = ph1[0:NA, 0, :]
            nc.tensor.matmul(pmu, lhsT=bmuf_sb[:, v, :], rhs=c, start=True, stop=False)
            nc.tensor.matmul(pmu, lhsT=wmub_sb, rhs=h2, start=False, stop=False)
            for e in range(3):
                nc.tensor.matmul(pmu, lhsT=wmud_sb[:, e, :], rhs=y3[:, e, :], start=False, stop=(e == 2))
            mu = act.tile([NA, NB], F32, tag="mu", name=f"mu_{t}")
            nc.scalar.copy(mu, pmu)
            nc.sync.dma_start(out=out[:, n0 : n0 + NB], in_=mu)

        LEAD = 2
        for i in range(nt + LEAD):
            if i < nt and i % 3 == 0:
                gate_triple(i)
            if i >= LEAD:
                heavy_phase(i - LEAD)


def _padk3(b):
    """Three zero-padded K=128 variants of bias-blend weights: variant v has
    the 4 live expert rows at partitions 32v..32v+3."""
    b = np.asarray(b, np.float32)  # [..., 4, M]
    shape = list(b.shape)
    shape[-2] = 128
    outs = []
    for v in range(3):
        o = np.zeros(shape, np.float32)
        o[..., 32 * v : 32 * v + 4, :] = b
        outs.append(o)
    return np.stack(outs, 0).astype(BF)  # [3, ..., 128, M]


def _gbop3(g):
    """[96,1] f32 with the 4 output-gate bias values at rows 32v..32v+3."""
    out = np.zeros((3 * GH, 1), np.float32)
    for v in range(3):
        out[32 * v : 32 * v + 4, 0] = g
    return out


def host_prep(inputs, bs=BS, ncores=NCORES):
    """Convert full f32 inputs to per-core in_maps (weights replicated)."""
    f32 = np.float32
    x = np.asarray(inputs["x"], f32)
    W1 = np.asarray(inputs["W1"], f32)
    b1 = np.asarray(inputs["b1"], f32)
    W2 = np.asarray(inputs["W2"], f32)
    b2 = np.asarray(inputs["b2"], f32)
    Wmu = np.asarray(inputs["Wmu"], f32)
    bmu = np.asarray(inputs["bmu"], f32)
    gW1 = np.asarray(inputs["gW1"], f32)
    gb1 = np.asarray(inputs["gb1"], f32)
    gW2 = np.asarray(inputs["gW2"], f32)
    gb2 = np.asarray(inputs["gb2"], f32)
    gWo = np.asarray(inputs["gWo"], f32)
    gbo = np.asarray(inputs["gbo"], f32)

    gw2_bf = gW2.astype(BF)
    gwo_bf = gWo.astype(BF)
    common = {
        "w1b": W1[3].reshape(2, 128, L1).astype(BF),
        "w1d": (W1[:3] - W1[3]).reshape(3, 2, 128, L1).astype(BF),
        "w2b": W2[3].reshape(2, 128, L2).astype(BF),
        "w2d": (W2[:3] - W2[3]).reshape(3, 2, 128, L2).astype(BF),
        "wmub": Wmu[3].astype(BF),
        "wmud": (Wmu[:3] - Wmu[3]).astype(BF),
        "gw1": gW1.reshape(2, 128, GH).astype(BF),
        "gw2x3": np.tile(gw2_bf, (3, 1)),
        "gwox3": np.tile(gwo_bf, (3, 1)),
        "b1f": _padk3(b1.reshape(4, 2, 128).transpose(1, 0, 2)),  # [3, 2, 128, 128]
        "b2f": _padk3(b2),                                        # [3, 128, 128]
        "bmuf": _padk3(bmu),                                      # [3, 128, 17]
        "gb1": np.tile(gb1, 3).reshape(3 * GH, 1).astype(f32),
        "gb2p": np.tile(gb2 - gw2_bf.astype(f32).sum(0), 3).reshape(3 * GH, 1).astype(f32),
        "gbop": _gbop3(gbo - gwo_bf.astype(f32).sum(0)),
    }
    xs = x.reshape(ncores, bs, D_IN)
    in_maps = []
    for i in range(ncores):
        m = dict(common)
        m["xt"] = xs[i].T.astype(BF)
        in_maps.append(m)
    return in_maps


_NC_CACHE = {}


def _get_nc():
    key = (BS, NCORES)
    if key not in _NC_CACHE:
        _NC_CACHE[key] = build_graph(BS, NCORES)
    return _NC_CACHE[key]


def kernel(**inputs):
    in_maps = host_prep(inputs)
    nc = _get_nc()
    res = run_bass_kernel_spmd(nc, in_maps, core_ids=list(range(NCORES)))
    outs = [m["out"] for m in res.results]  # each [17, BS] f32
    return np.concatenate([np.asarray(o, np.float32).T for o in outs], axis=0)


if __name__ == "__main__":
    # smoke build
    nc = build_graph(1024, 1)
    print("built ok")


# revision 64
# speedup vs baseline: 1.1683x; 1.0109x over previous
"""Trainium2 Bass kernel for nn_Actor (blended-MoE actor network).

Computation per batch row b:
    c     = softmax(gate(x_b))                          # [4] blend coeffs
    h1    = relu(sum_e c_e (x_b @ W1_e + b1_e))         # [256]
    h2    = relu(sum_e c_e (h1  @ W2_e + b2_e))         # [128]
    mu    = sum_e c_e (h2 @ Wmu_e + bmu_e)              # [17]

Strategy (pure data-parallel over 8 NeuronCores, 16384 rows/core):
  * Feature-on-partition layout: activations are [feat, batch] tiles, so
    expert weights load directly as matmul lhsT and layer biases are
    per-partition ACT biases.  x is transposed + cast to bf16 on the host.
  * Simplex trick: sum_e c_e A_e = A_3 + sum_{e<3} c_e' (A_e - A_3) with
    c' = c[:3]; saves 1/4 of the per-expert work.
  * Scale-input blending: sum_e c_e (x @ We) = x @ W3 + sum_e ((c_e*x) @ dWe)
    so the expert blend accumulates for free in PSUM.
  * ELU via exact identity elu(z)+1 = relu(z) + min(exp(z), 1); the +1 is
    folded into the next layer's bias on the host.
  * bf16 matmuls with fp32 PSUM accumulation.
"""

import sys

for _p in ("/opt/trn_rl_repo",):
    if _p not in sys.path:
        sys.path.append(_p)

import ml_dtypes
import numpy as np

import concourse.bass as bass
import concourse.mybir as mybir
import concourse.tile as tile
from concourse import bacc
from concourse.bass_utils import run_bass_kernel_spmd

AF = mybir.ActivationFunctionType
BF16 = mybir.dt.bfloat16
F32 = mybir.dt.float32
BF = ml_dtypes.bfloat16

NCORES = 8
B_FULL = 131072
BS = B_FULL // NCORES  # 16384 rows per core
NB = 512               # batch tile (matmul free dim)
D_IN = 256
L1 = 256
L2 = 128
NA = 17
GH = 32


def build_graph(bs: int = BS, num_devices: int = NCORES):
    """Build + compile the per-core Bass graph (same graph on all cores)."""
    nc = bacc.Bacc(
        "TRN2",
        target_bir_lowering=False,
        debug=False,
        enable_asserts=False,
        num_devices=num_devices,
    )
    d = {}

    def din(name, shape, dt):
        d[name] = nc.dram_tensor(name, shape, dt, kind="ExternalInput").ap()

    din("xt", [D_IN, bs], BF16)            # x shard, transposed
    din("w1b", [2, 128, L1], BF16)         # W1[3] as [k, part, m]
    din("w1d", [3, 2, 128, L1], BF16)      # W1[e]-W1[3]
    din("w2b", [2, 128, L2], BF16)
    din("w2d", [3, 2, 128, L2], BF16)
    din("wmub", [128, NA], BF16)
    din("wmud", [3, 128, NA], BF16)
    din("gw1", [2, 128, GH], BF16)
    din("gw2x3", [3 * GH, GH], BF16)       # gate W2 replicated at bases 0/32/64
    din("gwox3", [3 * GH, 4], BF16)
    # bias blend weights, zero-padded to K=128 with the live rows at
    # partition 32v (three variants, one per position in the gate triple)
    din("b1f", [3, 2, 128, 128], BF16)
    din("b2f", [3, 128, L2], BF16)
    din("bmuf", [3, 128, NA], BF16)
    din("gb1", [3 * GH, 1], F32)           # tiled x3
    din("gb2p", [3 * GH, 1], F32)          # (gb2 - colsum(gW2_bf16)) x3
    din("gbop", [3 * GH, 1], F32)          # gbo' at rows 32v..32v+3
    out = nc.dram_tensor("out", [NA, bs], F32, kind="ExternalOutput").ap()

    with tile.TileContext(nc) as tc:
        _body(tc, out, d, bs)
    nc.compile()
    return nc


def _body(tc, out, d, bs, dbg=None):
    nc = tc.nc
    nt = bs // NB

    with (
        tc.tile_pool(name="consts", bufs=1) as consts,
        tc.tile_pool(name="io", bufs=6) as io,
        tc.tile_pool(name="act", bufs=3) as act,
        tc.tile_pool(name="ps_g", bufs=1, space="PSUM") as ps_g,
        tc.tile_pool(name="ps_h1", bufs=2, space="PSUM") as ps_h1,
        tc.tile_pool(name="ps_h2", bufs=2, space="PSUM") as ps_h2,
        tc.tile_pool(name="dram", bufs=3, space="DRAM") as dram,
    ):
        # ---- load constants/weights (once) ----
        w1b_sb = consts.tile([128, 2, L1], BF16)
        w2b_sb = consts.tile([128, 2, L2], BF16)
        for k in range(2):
            nc.sync.dma_start(out=w1b_sb[:, k, :], in_=d["w1b"][k])
            nc.sync.dma_start(out=w2b_sb[:, k, :], in_=d["w2b"][k])
        w1d_sb = consts.tile([128, 3, 2, L1], BF16)
        w2d_sb = consts.tile([128, 3, 2, L2], BF16)
        for e in range(3):
            for k in range(2):
                nc.sync.dma_start(out=w1d_sb[:, e, k, :], in_=d["w1d"][e, k])
                nc.sync.dma_start(out=w2d_sb[:, e, k, :], in_=d["w2d"][e, k])
        wmub_sb = consts.tile([128, NA], BF16)
        nc.sync.dma_start(out=wmub_sb, in_=d["wmub"])
        wmud_sb = consts.tile([128, 3, NA], BF16)
        for e in range(3):
            nc.sync.dma_start(out=wmud_sb[:, e, :], in_=d["wmud"][e])
        gw1_sb = consts.tile([128, 2, GH], BF16)
        for k in range(2):
            nc.sync.dma_start(out=gw1_sb[:, k, :], in_=d["gw1"][k])
        gw2x3_sb = consts.tile([3 * GH, GH], BF16)
        nc.sync.dma_start(out=gw2x3_sb, in_=d["gw2x3"])
        gwox3_sb = consts.tile([3 * GH, 4], BF16)
        nc.sync.dma_start(out=gwox3_sb, in_=d["gwox3"])

        b1f_sb = consts.tile([128, 3, 2, 128], BF16)
        b2f_sb = consts.tile([128, 3, L2], BF16)
        bmuf_sb = consts.tile([128, 3, NA], BF16)
        for v in range(3):
            for m in range(2):
                nc.sync.dma_start(out=b1f_sb[:, v, m, :], in_=d["b1f"][v, m])
            nc.sync.dma_start(out=b2f_sb[:, v, :], in_=d["b2f"][v])
            nc.sync.dma_start(out=bmuf_sb[:, v, :], in_=d["bmuf"][v])
        gb1_sb = consts.tile([3 * GH, 1], F32)
        nc.sync.dma_start(out=gb1_sb, in_=d["gb1"])
        gb2p_sb = consts.tile([3 * GH, 1], F32)
        nc.sync.dma_start(out=gb2p_sb, in_=d["gb2p"])
        gbop_sb = consts.tile([3 * GH, 1], F32)
        nc.sync.dma_start(out=gbop_sb, in_=d["gbop"])

        ones96 = consts.tile([3 * GH, 1], BF16)
        nc.vector.memset(ones96, 1.0)

        # persistent zero-padded coefficient slots: c(t) occupies rows
        # 32(t%3)..+3 of column t%6; all other rows stay zero.  6 columns =
        # two gate-triples deep, since gate_triple(t0+3) is emitted before
        # heavy(t0+1)/heavy(t0+2) have consumed their coefficients.
        c_slots = consts.tile([128, 6, NB], BF16)
        nc.vector.memset(c_slots, 0.0)

        state = {}

        def gate_triple(t0):
            """Gate+softmax for supertiles t0..t0+2, fused across the triple:
            the three gate chains live at partition bases 0/32/64 of shared
            PSUM banks so each ACT/DVE elu op covers all three at once."""
            ts_ = [t for t in (t0, t0 + 1, t0 + 2) if t < nt]
            nm = len(ts_)
            P = GH * nm
            xts = {}
            for j, t in enumerate(ts_):
                n0 = t * NB
                xt = io.tile([128, 2, NB], BF16, tag="xt", name=f"xt_{t}")
                nc.sync.dma_start(out=xt[:, 0, :], in_=d["xt"][0:128, n0 : n0 + NB])
                nc.sync.dma_start(out=xt[:, 1, :], in_=d["xt"][128:256, n0 : n0 + NB])
                xts[t] = xt

            GA = ps_g.tile([128, NB], F32, tag="ga", name=f"GA_{t0}")
            GB = ps_g.tile([128, NB], F32, tag="gb", name=f"GB_{t0}")

            # layer 1 of the gate: pg1(t_j) at GA[32j:32j+32]
            for j, t in enumerate(ts_):
                pg1 = GA[GH * j : GH * (j + 1)]
                nc.tensor.matmul(pg1, lhsT=gw1_sb[:, 0, :], rhs=xts[t][:, 0, :], start=True, stop=False)
                nc.tensor.matmul(pg1, lhsT=gw1_sb[:, 1, :], rhs=xts[t][:, 1, :], start=False, stop=True)
            eg1 = act.tile([P, NB], BF16, tag="eg1", name=f"eg1_{t0}")
            nc.scalar.activation(eg1, GA[0:P], AF.Exp, bias=gb1_sb[0:P])
            rg1 = act.tile([P, NB], BF16, tag="rg1", name=f"rg1_{t0}")
            nc.vector.tensor_scalar(
                rg1, GA[0:P], gb1_sb[0:P], 0.0,
                op0=mybir.AluOpType.add, op1=mybir.AluOpType.max,
            )
            g1 = act.tile([P, NB], BF16, tag="g1", name=f"g1_{t0}")
            nc.vector.scalar_tensor_tensor(
                g1, eg1, 1.0, rg1, op0=mybir.AluOpType.min, op1=mybir.AluOpType.add
            )
            # layer 2: K=32 row-group matmuls run concurrently
            for j in range(nm):
                s = slice(GH * j, GH * (j + 1))
                nc.tensor.matmul(GB[s], lhsT=gw2x3_sb[s, :], rhs=g1[s, :], start=True, stop=True)
            eg2 = act.tile([P, NB], BF16, tag="eg2", name=f"eg2_{t0}")
            nc.scalar.activation(eg2, GB[0:P], AF.Exp, bias=gb2p_sb[0:P])
            rg2 = act.tile([P, NB], BF16, tag="rg2", name=f"rg2_{t0}")
            nc.vector.tensor_scalar(
                rg2, GB[0:P], gb2p_sb[0:P], 0.0,
                op0=mybir.AluOpType.add, op1=mybir.AluOpType.max,
            )
            g2 = act.tile([P, NB], BF16, tag="g2", name=f"g2_{t0}")
            nc.vector.scalar_tensor_tensor(
                g2, eg2, 1.0, rg2, op0=mybir.AluOpType.min, op1=mybir.AluOpType.add
            )
            # logits at GA[32j:32j+4] (reusing GA after eg1/rg1 read it)
            expv = act.tile([GH * 2 + 4, NB], BF16, tag="expv", name=f"expv_{t0}")
            for j, t in enumerate(ts_):
                s4 = slice(GH * j, GH * j + 4)
                nc.tensor.matmul(
                    GA[s4], lhsT=gwox3_sb[GH * j : GH * (j + 1), :],
                    rhs=g2[GH * j : GH * (j + 1), :], start=True, stop=True,
                )
                nc.scalar.activation(expv[s4], GA[s4], AF.Exp, bias=gbop_sb[s4])
                # denominator at GB[0:1] (base 0: reciprocal_approx_fast
                # gives wrong results at nonzero base partitions on HW)
                nc.tensor.matmul(
                    GB[0:1], lhsT=ones96[s4], rhs=expv[s4], start=True, stop=True
                )
                rden = act.tile([1, NB], F32, tag=f"rden{j}", name=f"rden_{t}")
                nc.vector.reciprocal_approx_fast(out=rden, in_=GB[0:1])
                # broadcast 1/den to the 4 coeff rows via DRAM bounce
                rden_dram = dram.tile([1, NB], F32, tag="rden_dram", name=f"rden_dram_{t}")
                nc.sync.dma_start(out=rden_dram, in_=rden)
                r4sb = act.tile([GH * 2 + 4, NB], F32, tag="r4sb", name=f"r4sb_{t}")
                nc.sync.dma_start(out=r4sb[s4], in_=rden_dram.to_broadcast([4, NB]))
                cs = c_slots[:, t % 6, :]
                nc.vector.tensor_mul(cs[s4], expv[s4], r4sb[s4])

                # Broadcast c rows to one [128, 3, 2, NB] tile via DRAM bounce
                c_dram = dram.tile([1, 3, 2, NB], BF16, tag="c_dram", name=f"c_dram_{t}")
                nc.gpsimd.dma_start(out=c_dram[0:1, :, 0, :], in_=cs[GH * j : GH * j + 3, :])
                nc.gpsimd.dma_start(out=c_dram[0:1, :, 1, :], in_=cs[GH * j : GH * j + 3, :])
                cb = act.tile([128, 3, 2, NB], BF16, tag="cb", name=f"cb_{t}", bufs=6)
                nc.gpsimd.dma_start(out=cb, in_=c_dram.to_broadcast([128, 3, 2, NB]))

                # y1[e,k] = c'_e * x_k for all (e,k) in ONE DVE op
                xt = xts[t]
                xt3 = bass.AP(
                    tensor=xt.tensor, offset=xt.offset,
                    ap=[list(xt.ap[0]), [0, 3]] + [list(p) for p in xt.ap[1:]],
                )
                y1 = io.tile([128, 3, 2, NB], BF16, tag="y1", name=f"y1_{t}")
                nc.vector.tensor_mul(y1, xt3, cb)
                state[t] = (xt, cs, cb, y1)

            if dbg is not None and t0 == 0:
                nc.sync.dma_start(out=dbg["g1"], in_=g1)
                nc.sync.dma_start(out=dbg["g2"], in_=g2)
                nc.sync.dma_start(out=dbg["expv"], in_=expv)
                nc.sync.dma_start(out=dbg["c"], in_=c_slots)

        def heavy_phase(t):
            n0 = t * NB
            xt, c, cb, y1 = state.pop(t)
            v = t % 3

            # ---- layer 1 (one [128, 2, NB] PSUM tile spanning 2 banks) ----
            ph1 = ps_h1.tile([128, 2, NB], F32, tag="h1", name=f"ph1_{t}")
            for m in range(2):
                pm = ph1[:, m, :]
                ms = slice(m * 128, (m + 1) * 128)
                nc.tensor.matmul(pm, lhsT=b1f_sb[:, v, m, :], rhs=c, start=True, stop=False)
                nc.tensor.matmul(pm, lhsT=w1b_sb[:, 0, ms], rhs=xt[:, 0, :], start=False, stop=False)
                nc.tensor.matmul(pm, lhsT=w1b_sb[:, 1, ms], rhs=xt[:, 1, :], start=False, stop=False)
                for e in range(3):
                    for k in range(2):
                        nc.tensor.matmul(
                            pm, lhsT=w1d_sb[:, e, k, ms], rhs=y1[:, e, k, :],
                            start=False, stop=(e == 2 and k == 1),
                        )
            h1 = act.tile([128, 2, NB], BF16, tag="h1", name=f"h1_{t}")
            nc.scalar.activation(h1, ph1, AF.Relu)

            # ---- layer 2 (y2 in one DVE op via stride-0 expert axis) ----
            h13 = bass.AP(
                tensor=h1.tensor, offset=h1.offset,
                ap=[list(h1.ap[0]), [0, 3]] + [list(p) for p in h1.ap[1:]],
            )
            y2 = act.tile([128, 3, 2, NB], BF16, tag="y2", name=f"y2_{t}")
            nc.vector.tensor_mul(y2, h13, cb)
            ph2 = ps_h2.tile([L2, NB], F32, tag="h2", name=f"ph2_{t}")
            nc.tensor.matmul(ph2, lhsT=b2f_sb[:, v, :], rhs=c, start=True, stop=False)
            nc.tensor.matmul(ph2, lhsT=w2b_sb[:, 0, :], rhs=h1[:, 0, :], start=False, stop=False)
            nc.tensor.matmul(ph2, lhsT=w2b_sb[:, 1, :], rhs=h1[:, 1, :], start=False, stop=False)
            for e in range(3):
                for k in range(2):
                    nc.tensor.matmul(
                        ph2, lhsT=w2d_sb[:, e, k, :], rhs=y2[:, e, k, :],
                        start=False, stop=(e == 2 and k == 1),
                    )
            h2 = act.tile([L2, NB], BF16, tag="h2s", name=f"h2_{t}")
            nc.scalar.activation(h2, ph2, AF.Relu)

            # ---- output head (PSUM: reuse partitions 0:17 of ph1 bank 0) ----
            h23 = bass.AP(
                tensor=h2.tensor, offset=h2.offset,
                ap=[list(h2.ap[0]), [0, 3]] + [list(p) for p in h2.ap[1:]],
            )
            y3 = act.tile([L2, 3, NB], BF16, tag="y3", name=f"y3_{t}")
            nc.vector.tensor_mul(y3, h23, cb[:L2, :, 0, :])
            pmu = ph1[0:NA, 0, :]
            nc.tensor.matmul(pmu, lhsT=bmuf_sb[:, v, :], rhs=c, start=True, stop=False)
            nc.tensor.matmul(pmu, lhsT=wmub_sb, rhs=h2, start=False, stop=False)
            for e in range(3):
                nc.tensor.matmul(pmu, lhsT=wmud_sb[:, e, :], rhs=y3[:, e, :], start=False, stop=(e == 2))
            mu = act.tile([NA, NB], F32, tag="mu", name=f"mu_{t}")
            nc.scalar.copy(mu, pmu)
            nc.sync.dma_start(out=out[:, n0 : n0 + NB], in_=mu)

        LEAD = 3
        for i in range(nt + LEAD):
            if i < nt and i % 3 == 0:
                gate_triple(i)
            if i >= LEAD:
                heavy_phase(i - LEAD)


def _padk3(b):
    """Three zero-padded K=128 variants of bias-blend weights: variant v has
    the 4 live expert rows at partitions 32v..32v+3."""
    b = np.asarray(b, np.float32)  # [..., 4, M]
    shape = list(b.shape)
    shape[-2] = 128
    outs = []
    for v in range(3):
        o = np.zeros(shape, np.float32)
        o[..., 32 * v : 32 * v + 4, :] = b
        outs.append(o)
    return np.stack(outs, 0).astype(BF)  # [3, ..., 128, M]


def _gbop3(g):
    """[96,1] f32 with the 4 output-gate bias values at rows 32v..32v+3."""
    out = np.zeros((3 * GH, 1), np.float32)
    for v in range(3):
        out[32 * v : 32 * v + 4, 0] = g
    return out


def host_prep(inputs, bs=BS, ncores=NCORES):
    """Convert full f32 inputs to per-core in_maps (weights replicated)."""
    f32 = np.float32
    x = np.asarray(inputs["x"], f32)
    W1 = np.asarray(inputs["W1"], f32)
    b1 = np.asarray(inputs["b1"], f32)
    W2 = np.asarray(inputs["W2"], f32)
    b2 = np.asarray(inputs["b2"], f32)
    Wmu = np.asarray(inputs["Wmu"], f32)
    bmu = np.asarray(inputs["bmu"], f32)
    gW1 = np.asarray(inputs["gW1"], f32)
    gb1 = np.asarray(inputs["gb1"], f32)
    gW2 = np.asarray(inputs["gW2"], f32)
    gb2 = np.asarray(inputs["gb2"], f32)
    gWo = np.asarray(inputs["gWo"], f32)
    gbo = np.asarray(inputs["gbo"], f32)

    gw2_bf = gW2.astype(BF)
    gwo_bf = gWo.astype(BF)
    common = {
        "w1b": W1[3].reshape(2, 128, L1).astype(BF),
        "w1d": (W1[:3] - W1[3]).reshape(3, 2, 128, L1).astype(BF),
        "w2b": W2[3].reshape(2, 128, L2).astype(BF),
        "w2d": (W2[:3] - W2[3]).reshape(3, 2, 128, L2).astype(BF),
        "wmub": Wmu[3].astype(BF),
        "wmud": (Wmu[:3] - Wmu[3]).astype(BF),
        "gw1": gW1.reshape(2, 128, GH).astype(BF),
        "gw2x3": np.tile(gw2_bf, (3, 1)),
        "gwox3": np.tile(gwo_bf, (3, 1)),
        "b1f": _padk3(b1.reshape(4, 2, 128).transpose(1, 0, 2)),  # [3, 2, 128, 128]
        "b2f": _padk3(b2),                                        # [3, 128, 128]
        "bmuf": _padk3(bmu),                                      # [3, 128, 17]
        "gb1": np.tile(gb1, 3).reshape(3 * GH, 1).astype(f32),
        "gb2p": np.tile(gb2 - gw2_bf.astype(f32).sum(0), 3).reshape(3 * GH, 1).astype(f32),
        "gbop": _gbop3(gbo - gwo_bf.astype(f32).sum(0)),
    }
    xs = x.reshape(ncores, bs, D_IN)
    in_maps = []
    for i in range(ncores):
        m = dict(common)
        m["xt"] = xs[i].T.astype(BF)
        in_maps.append(m)
    return in_maps


_NC_CACHE = {}


def _get_nc():
    key = (BS, NCORES)
    if key not in _NC_CACHE:
        _NC_CACHE[key] = build_graph(BS, NCORES)
    return _NC_CACHE[key]


def kernel(**inputs):
    in_maps = host_prep(inputs)
    nc = _get_nc()
    res = run_bass_kernel_spmd(nc, in_maps, core_ids=list(range(NCORES)))
    outs = [m["out"] for m in res.results]  # each [17, BS] f32
    return np.concatenate([np.asarray(o, np.float32).T for o in outs], axis=0)


if __name__ == "__main__":
    # smoke build
    nc = build_graph(1024, 1)
    print("built ok")
